# revision 1
# baseline (speedup 1.0000x reference)
"""Trainium2 Bass kernel for nn_NnBoard768 (sparse embedding lookup NNUE head).

Strategy (data-parallel over batch, 8 cores):
  - Each core handles 1024 of the 8192 batch rows. Batch row b sits at
    SBUF partition b%128, free-slot b//128.
  - The feature table is shipped to HBM premultiplied by TSCALE in fp8
    (e4m3), quartering gather traffic; the PE accumulates in fp32 and its
    identity is diag(1/TSCALE), so only the table entries round.
  - Rows are fetched with the TIE-accelerated `dma_gather` instruction
    (4 SWDGE queues; descriptor generation is the main serial cost).
    Its indices are int16, so each (side, k) gather runs as two passes:
    pass A covers table rows < SA, pass B covers the rest (rebased).
    Out-of-pass slots read a zero row from a 4096-row zero block in
    front of the pass base (spread across rows to avoid hammering one
    HBM channel), so every index is valid, every slot is written, and
    invalid-slot reads contribute nothing to the sum.
  - The sum over the 32 active features runs on the tensor engine:
    scaled-identity matmuls accumulate every gathered tile into PSUM.
  - Epilogue on DVE/ACT: +b_ft, clip(0,1), dot with W_out, +b_out, sigmoid.
"""

import sys

sys.path.insert(0, "/opt/trn_rl_repo")

import numpy as np
import ml_dtypes

from concourse import bacc, bass, mybir
from concourse.masks import make_identity
import concourse.tile as tile
from concourse.bass_utils import run_bass_kernel_spmd

P = 128          # SBUF partitions
K = 32           # nnz (active features per position)
J = 8            # batch slots per partition
F = 512          # feature-table output width
NCORES = 8
BPC = P * J      # batch rows per core (1024)
FT_IN = 40960
ZPAD = 4096      # zero rows in front of each pass base (junk reads spread
                 # across them instead of hammering one HBM row)
SA = 32768 - ZPAD          # rows < SA -> pass A; rest -> pass B
# device table layout: [ZA(ZPAD), W[0:SA], ZB(ZPAD), W[SA:]]
VDEV = FT_IN + 2 * ZPAD
BOFF = ZPAD + SA           # byte row where the B view starts (= 32768)
S16 = BPC // 16  # columns of the 16-partition-wrapped index tile (64)

f32 = mybir.dt.float32
bf16 = mybir.dt.bfloat16
i16 = mybir.dt.int16
Alu = mybir.AluOpType

TDT = mybir.dt.float8e4            # gathered-table dtype on device
TDT_NP = ml_dtypes.float8_e4m3     # host equivalent
TSCALE = 64.0                      # host premultiplier; PE identity = 1/TSCALE

GBUFS = 8        # in-flight gather tiles per (A/B) tag
NQ = 4           # SWDGE descriptor-generation queues (parallel on HW)


def _build(fast: bool):
    nc = bacc.Bacc("TRN2", target_bir_lowering=False, debug=False, num_devices=NCORES,
                   num_swdge_queues=NQ)

    idx_in = {}
    for side in ("stm", "nstm"):
        for part in ("a", "b"):
            idx_in[(side, part)] = nc.dram_tensor(
                f"i{part}_{side}", [P, K, S16], i16, kind="ExternalInput"
            )
    wft = nc.dram_tensor("w_ft", [VDEV, F], TDT, kind="ExternalInput")
    bft = nc.dram_tensor("bft", [P, F], f32, kind="ExternalInput")
    w1 = nc.dram_tensor("w1", [P, F], f32, kind="ExternalInput")
    w2 = nc.dram_tensor("w2", [P, F], f32, kind="ExternalInput")
    bout = nc.dram_tensor("bout", [P, 1], f32, kind="ExternalInput")
    if not fast:
        vals = nc.dram_tensor("vals", [P, K, J], f32, kind="ExternalInput")
    out = nc.dram_tensor("out", [P, J], f32, kind="ExternalOutput")

    gbufs = GBUFS if fast else 2
    with tile.TileContext(nc) as tc:
        with tc.tile_pool(name="sbuf", bufs=1) as pool, \
             tc.tile_pool(name="gather", bufs=gbufs) as gpool, \
             tc.tile_pool(name="psum", bufs=1, space="PSUM") as ppool:
            idx_sb = {}
            for side_i, side in enumerate(("stm", "nstm")):
                for part in ("a", "b"):
                    t = pool.tile(
                        [P, K, S16], i16,
                        tag=f"i{part}{side_i}", name=f"i{part}_{side}_sb",
                    )
                    nc.sync.dma_start(out=t[:], in_=idx_in[(side, part)][:])
                    idx_sb[(side_i, part)] = t
            bft_sb = pool.tile([P, F], f32, tag="bft", name="bft_sb")
            nc.sync.dma_start(out=bft_sb[:], in_=bft[:])
            w_sb = [
                pool.tile([P, F], f32, tag="w1", name="w1_sb"),
                pool.tile([P, F], f32, tag="w2", name="w2_sb"),
            ]
            nc.sync.dma_start(out=w_sb[0][:], in_=w1[:])
            nc.sync.dma_start(out=w_sb[1][:], in_=w2[:])
            bout_sb = pool.tile([P, 1], f32, tag="bout", name="bout_sb")
            nc.sync.dma_start(out=bout_sb[:], in_=bout[:])
            ident = pool.tile([P, P], TDT, tag="ident", name="ident")
            make_identity(nc, ident[:])
            nc.vector.tensor_scalar_mul(ident[:], ident[:], 1.0 / TSCALE)
            if not fast:
                vals_sb = pool.tile([P, K, J], f32, tag="vals", name="vals_sb")
                nc.sync.dma_start(out=vals_sb[:], in_=vals[:])

            def bcast(t2d):  # [P, F] -> [P, J, F] AP (stride-0 over J)
                return t2d[:].rearrange("p (j f) -> p j f", j=1).broadcast_to([P, J, F])

            z = [
                pool.tile([P, J], f32, tag=f"z{side}", name=f"z{side}")
                for side in range(2)
            ]
            for side in range(2):
                if fast:
                    acc = ppool.tile(
                        [P, J, F], f32, space="PSUM", tag="acc", name=f"acc{side}"
                    )
                else:
                    acc = pool.tile([P, J, F], f32, tag=f"sacc{side}", name=f"sacc{side}")
                for k in range(K):
                    ga = gpool.tile([P, J, F], TDT, tag="ga", name="ga")
                    gb = gpool.tile([P, J, F], TDT, tag="gb", name="gb")
                    qa = (side * 2 * K + 2 * k) % NQ
                    nc.gpsimd.dma_gather(
                        ga[:], wft[:, :], idx_sb[(side, "a")][:, k, :],
                        num_idxs=BPC, num_idxs_reg=BPC, elem_size=F,
                        queue_num=qa,
                    )
                    nc.gpsimd.dma_gather(
                        gb[:], wft[BOFF:, :], idx_sb[(side, "b")][:, k, :],
                        num_idxs=BPC, num_idxs_reg=BPC, elem_size=F,
                        queue_num=(qa + 1) % NQ,
                    )
                    if fast:
                        for j in range(J):
                            nc.tensor.matmul(
                                acc[:, j, :], ident[:], ga[:, j, :],
                                start=(k == 0), stop=False,
                            )
                        for j in range(J):
                            nc.tensor.matmul(
                                acc[:, j, :], ident[:], gb[:, j, :],
                                start=False, stop=(k == K - 1),
                            )
                    else:
                        vb = (
                            vals_sb[:, k, :]
                            .rearrange("p (j f) -> p j f", f=1)
                            .broadcast_to([P, J, F])
                        )
                        t = gpool.tile([P, J, F], f32, tag="t", name="t")
                        nc.vector.tensor_tensor(out=t[:], in0=ga[:], in1=gb[:], op=Alu.add)
                        if k == 0:
                            nc.vector.tensor_tensor(out=acc[:], in0=t[:], in1=vb, op=Alu.mult)
                        else:
                            nc.vector.tensor_tensor(out=t[:], in0=t[:], in1=vb, op=Alu.mult)
                            nc.vector.tensor_tensor(out=acc[:], in0=acc[:], in1=t[:], op=Alu.add)

                # epilogue: h = clip(acc + b_ft, 0, 1) * w_side; z = sum_f h
                h = pool.tile([P, J, F], f32, tag=f"h{side}", name=f"h{side}")
                nc.vector.tensor_tensor(out=h[:], in0=acc[:], in1=bcast(bft_sb), op=Alu.add)
                nc.vector.tensor_scalar(
                    out=h[:], in0=h[:], scalar1=0.0, scalar2=1.0,
                    op0=Alu.max, op1=Alu.min,
                )
                nc.vector.tensor_tensor(out=h[:], in0=h[:], in1=bcast(w_sb[side]), op=Alu.mult)
                nc.vector.tensor_reduce(
                    out=z[side][:], in_=h[:], axis=mybir.AxisListType.X, op=Alu.add
                )
            nc.vector.tensor_tensor(out=z[0][:], in0=z[0][:], in1=z[1][:], op=Alu.add)
            out_sb = pool.tile([P, J], f32, tag="out", name="out_sb")
            nc.scalar.activation(
                out=out_sb[:],
                in_=z[0][:],
                func=mybir.ActivationFunctionType.Sigmoid,
                bias=bout_sb[:, :1],
            )
            nc.sync.dma_start(out=out.ap(), in_=out_sb[:])

    nc.compile()
    return nc


_cache = {}


def _get(fast: bool):
    if fast not in _cache:
        _cache[fast] = _build(fast)
    return _cache[fast]


def _prep_table(W_ft: np.ndarray) -> np.ndarray:
    """f32 [40960, 512] -> TSCALE-premultiplied TDT [VDEV, 512]: zero pad
    blocks ahead of each pass segment so junk reads land on spread-out
    zero rows."""
    w = np.zeros((VDEV, F), dtype=TDT_NP)
    w[ZPAD:ZPAD + SA] = (W_ft[:SA] * TSCALE).astype(TDT_NP)
    w[BOFF + ZPAD:] = (W_ft[SA:] * TSCALE).astype(TDT_NP)
    return w


def _prep_idx(idx_core: np.ndarray):
    """[1024, 32] int32 -> (A, B) int16 arrays of shape [128, 32, 64].

    Index g (= batch row b) for feature-slot k lives at partition g%16,
    column g//16 (replicated across the 8 16-partition groups).
    Out-of-pass slots read a (spread) zero row from the pass's ZPAD
    block, so every index is valid and every slot is written.
    """
    t3 = idx_core.astype(np.int64).reshape(S16, 16, K).transpose(2, 1, 0)  # [K,16,S16]
    spread = (np.arange(t3.size, dtype=np.int64).reshape(t3.shape) * 37) % ZPAD
    a = np.where(t3 < SA, t3 + ZPAD, spread).astype(np.int16)
    b = np.where(t3 >= SA, t3 - SA + ZPAD, spread).astype(np.int16)
    a = np.ascontiguousarray(np.tile(a, (1, 8, 1)).transpose(1, 0, 2))  # [128,K,S16]
    b = np.ascontiguousarray(np.tile(b, (1, 8, 1)).transpose(1, 0, 2))
    return a, b


def kernel(stm_indices, nstm_indices, values, W_ft, b_ft, W_out, b_out, _trace=False):
    stm_indices = np.asarray(stm_indices)
    nstm_indices = np.asarray(nstm_indices)
    values = np.asarray(values, dtype=np.float32)
    W_ft = np.ascontiguousarray(np.asarray(W_ft, dtype=np.float32))
    b_ft = np.asarray(b_ft, dtype=np.float32)
    W_out = np.asarray(W_out, dtype=np.float32)
    b_out = np.asarray(b_out, dtype=np.float32)

    fast = bool(np.all(values == 1.0))
    nc = _get(fast)

    w_dev = _prep_table(W_ft)
    bft_rep = np.ascontiguousarray(np.broadcast_to(b_ft, (P, F)).astype(np.float32))
    w1_rep = np.ascontiguousarray(np.broadcast_to(W_out[:F, 0], (P, F)).astype(np.float32))
    w2_rep = np.ascontiguousarray(np.broadcast_to(W_out[F:, 0], (P, F)).astype(np.float32))
    bout_rep = np.full((P, 1), b_out[0], dtype=np.float32)

    in_maps = []
    for c in range(NCORES):
        sl = slice(c * BPC, (c + 1) * BPC)
        m = {
            "w_ft": w_dev,
            "bft": bft_rep,
            "w1": w1_rep,
            "w2": w2_rep,
            "bout": bout_rep,
        }
        for side, arr in (("stm", stm_indices), ("nstm", nstm_indices)):
            a, b = _prep_idx(arr[sl])
            m[f"ia_{side}"] = a
            m[f"ib_{side}"] = b
        if not fast:
            # vals[p, k, j] = values[j*128 + p, k]
            m["vals"] = np.ascontiguousarray(
                values[sl].reshape(J, P, K).transpose(1, 2, 0) / TSCALE
            )
        in_maps.append(m)

    res = run_bass_kernel_spmd(
        nc, in_maps, core_ids=list(range(NCORES)), trace=_trace
    )
    # out[p, j] holds batch row j*128 + p
    out = np.concatenate(
        [res.results[c]["out"].T.reshape(BPC) for c in range(NCORES)]
    ).reshape(8192, 1)
    if _trace:
        return out, res
    return out



# revision 8
# speedup vs baseline: 1.8244x; 1.8244x over previous
"""Trainium2 Bass kernel for nn_NnBoard768 (sparse embedding-lookup NNUE head).

Strategy (data-parallel over batch, 8 cores, input-specialized compile):
  - Each core handles 1024 of the 8192 batch rows; row b sits at SBUF
    partition b%128, free-slot b//128.
  - The feature table is REMAPPED per core: a core references only ~32.7k
    unique rows, so its table copy holds exactly those rows (fp8 e4m3,
    premultiplied by TSCALE) with device ids 0..nA-1 < 32704 — directly
    addressable by the int16 indices of the TIE `dma_gather` instruction.
    The (rare, data-dependent) overflow rows live in a small "B" region
    addressed by a second gather view.  This removes the two-pass
    zero-row-junk scheme entirely: every gathered descriptor is a needed
    row, halving HBM gather traffic vs. the two-pass kernel.
  - Per-position indices are sorted ascending, so gather k covers a narrow
    band of table rows (HBM locality) and overflow ids cluster at the tail.
  - Accumulation over the 32 active features runs on the tensor engine with
    fp8 DoubleRow matmuls (2 gathered tiles per instruction, 0.5 cyc/row)
    against a stacked scaled identity; b_ft is added by one bf16 matmul.
  - Work is split into 4 PSUM phases (side x batch-half, 4 banks each,
    double-buffered) so each phase's DVE epilogue (clip to [0,1], fused
    multiply+reduce against W_out) overlaps the next phase's gathers.
"""

import sys

sys.path.insert(0, "/opt/trn_rl_repo")

import numpy as np
import ml_dtypes

from concourse import bacc, bass, mybir
from concourse.masks import make_identity
import concourse.tile as tile
from concourse.bass_utils import run_bass_kernel_spmd

P = 128          # SBUF partitions
K = 32           # nnz (active features per position)
J = 8            # batch slots per partition per core
JH = 4           # j-blocks per PSUM phase
F = 512          # feature-table output width
NCORES = 8
BPC = P * J      # batch rows per core (1024)
FT_IN = 40960

NA_CAP = 32704           # direct ids 0..32703
AZ0 = 32704              # 64 zero rows for A-junk: ids 32704..32767
BBASE = 32768            # overflow ("B") region base device row
BCAP = 576               # max overflow rows
BZ0 = 576                # B-junk ids 576..639 (zero rows)
VDEV = BBASE + BCAP + 64  # 33408 device rows

f32 = mybir.dt.float32
bf16 = mybir.dt.bfloat16
i16 = mybir.dt.int16
f8 = mybir.dt.float8e4
F8_NP = ml_dtypes.float8_e4m3
BF16_NP = ml_dtypes.bfloat16
Alu = mybir.AluOpType
DR = mybir.MatmulPerfMode.DoubleRow

TSCALE = 64.0    # host premultiplier; PE identity = 1/TSCALE
NQ = 4           # SWDGE descriptor-generation queues
GBUFS = 8
NPH = 4          # phases: (side, half)
NGA = K // 2     # 16 A-gathers per phase (k-pairs), 1024 idxs each
SA16 = (2 * JH * P) // 16   # 64 idx cols per A-gather

WIDE_MM = False  # wide (4-bank) matmul fails the s3d3 ISA check; use per-bank


def _build(nbslots: int):
    nc = bacc.Bacc("TRN2", target_bir_lowering=False, debug=False,
                   num_devices=NCORES, num_swdge_queues=NQ)

    wft = nc.dram_tensor("w_ft", [VDEV, F], f8, kind="ExternalInput")
    idxa_in = nc.dram_tensor("idxa", [P, NPH * NGA, SA16], i16,
                             kind="ExternalInput")
    if nbslots:
        idxb_in = nc.dram_tensor("idxb", [P, NPH, nbslots * 32], i16,
                                 kind="ExternalInput")
    bias_in = nc.dram_tensor("bias", [P, JH, F], bf16, kind="ExternalInput")
    wout_in = nc.dram_tensor("wout", [P, 2, F], bf16, kind="ExternalInput")
    bout_in = nc.dram_tensor("bout", [P, 1], f32, kind="ExternalInput")
    out = nc.dram_tensor("out", [P, J], f32, kind="ExternalOutput")

    qn = 0
    with tile.TileContext(nc) as tc:
        with tc.tile_pool(name="sbuf", bufs=1) as pool, \
             tc.tile_pool(name="gather", bufs=GBUFS) as gpool, \
             tc.tile_pool(name="psum", bufs=2, space="PSUM") as ppool:
            idxa = pool.tile([P, NPH * NGA, SA16], i16, tag="idxa")
            nc.sync.dma_start(out=idxa[:], in_=idxa_in[:])
            if nbslots:
                idxb = pool.tile([P, NPH, nbslots * 32], i16, tag="idxb")
                nc.sync.dma_start(out=idxb[:], in_=idxb_in[:])
            bias_sb = pool.tile([P, JH, F], bf16, tag="bias")
            nc.sync.dma_start(out=bias_sb[:], in_=bias_in[:])
            wout_sb = pool.tile([P, 2, F], bf16, tag="wout")
            nc.sync.dma_start(out=wout_sb[:], in_=wout_in[:])
            bout_sb = pool.tile([P, 1], f32, tag="bout")
            nc.sync.dma_start(out=bout_sb[:], in_=bout_in[:])

            identW = pool.tile([P, 2, P], f8, tag="identW")
            for i in range(2):
                make_identity(nc, identW[:, i, :])
            nc.vector.tensor_scalar_mul(identW[:], identW[:], 1.0 / TSCALE)
            identB = pool.tile([P, P], bf16, tag="identB")
            make_identity(nc, identB[:])

            z = [pool.tile([P, J], f32, tag=f"z{s}", name=f"z{s}")
                 for s in range(2)]
            prod = pool.tile([P, JH, F], bf16, tag="prod")

            for ph in range(NPH):
                s, hf = divmod(ph, 2)
                acc = ppool.tile([P, JH, F], f32, tag="acc", name=f"acc{ph}")
                for g in range(NGA):
                    ga = gpool.tile([P, 2, JH, F], f8, tag="ga",
                                    name=f"ga{ph}_{g}")
                    nc.gpsimd.dma_gather(
                        ga[:].rearrange("p i j f -> p (i j) f"),
                        wft[:, :], idxa[:, ph * NGA + g, :],
                        num_idxs=2 * JH * P, num_idxs_reg=2 * JH * P,
                        elem_size=F, queue_num=qn % NQ)
                    qn += 1
                    if WIDE_MM:
                        nc.tensor.matmul(acc[:], identW[:], ga[:],
                                         start=(g == 0), stop=False,
                                         perf_mode=DR)
                    else:
                        for jh in range(JH):
                            nc.tensor.matmul(
                                acc[:, jh, :], identW[:], ga[:, :, jh, :],
                                start=(g == 0), stop=False, perf_mode=DR)
                if nbslots:
                    gb = gpool.tile([P, nbslots, JH, F], f8, tag="gb",
                                    name=f"gb{ph}")
                    nc.gpsimd.dma_gather(
                        gb[:].rearrange("p m j f -> p (m j) f"),
                        wft[BBASE:, :], idxb[:, ph, :],
                        num_idxs=nbslots * JH * P,
                        num_idxs_reg=nbslots * JH * P,
                        elem_size=F, queue_num=qn % NQ)
                    qn += 1
                    for m in range(nbslots):
                        if WIDE_MM:
                            nc.tensor.matmul(acc[:], identW[:, 0, :],
                                             gb[:, m, :, :],
                                             start=False, stop=False)
                        else:
                            for jh in range(JH):
                                nc.tensor.matmul(
                                    acc[:, jh, :], identW[:, 0, :],
                                    gb[:, m, jh, :],
                                    start=False, stop=False)
                if WIDE_MM:
                    nc.tensor.matmul(acc[:], identB[:], bias_sb[:],
                                     start=False, stop=True)
                else:
                    for jh in range(JH):
                        nc.tensor.matmul(acc[:, jh, :], identB[:],
                                         bias_sb[:, jh, :],
                                         start=False, stop=(True))

                h = pool.tile([P, JH, F], bf16, tag="h", name=f"h{ph}")
                nc.vector.tensor_scalar(
                    out=h[:], in0=acc[:], scalar1=0.0, scalar2=1.0,
                    op0=Alu.max, op1=Alu.min)
                wb = (wout_sb[:, s, :]
                      .rearrange("p (j f) -> p j f", j=1)
                      .broadcast_to([P, JH, F]))
                nc.vector.tensor_tensor(out=prod[:], in0=h[:], in1=wb,
                                        op=Alu.mult)
                nc.vector.tensor_reduce(
                    out=z[s][:, hf * JH:(hf + 1) * JH], in_=prod[:],
                    axis=mybir.AxisListType.X, op=Alu.add)

            nc.vector.tensor_tensor(out=z[0][:], in0=z[0][:], in1=z[1][:],
                                    op=Alu.add)
            out_sb = pool.tile([P, J], f32, tag="out")
            nc.scalar.activation(
                out=out_sb[:], in_=z[0][:],
                func=mybir.ActivationFunctionType.Sigmoid,
                bias=bout_sb[:, :1])
            nc.sync.dma_start(out=out.ap(), in_=out_sb[:])

    nc.compile()
    return nc


_cache = {}


def _get(nbslots: int):
    if nbslots not in _cache:
        _cache[nbslots] = _build(nbslots)
    return _cache[nbslots]


def _wrap16(vec: np.ndarray) -> np.ndarray:
    """int vector -> [128, len//16] int16 tile (idx i at partition i%16,
    col i//16, replicated across the 8 16-partition groups)."""
    arr = vec.reshape(-1, 16).T.astype(np.int16)
    return np.tile(arr, (8, 1))


def _prep_core(stm_c: np.ndarray, nstm_c: np.ndarray, W_ft: np.ndarray):
    """Remap one core's referenced table rows into the device layout and
    build its gather index tiles.

    Returns (w_dev [VDEV, F] fp8, idxa [128, 64, 64] i16 pieces per
    (phase, gather), bvecs per phase (device-B-relative, -junk-filled),
    maxb)."""
    both = np.stack([stm_c, nstm_c]).astype(np.int64)   # [2, 1024, 32]
    counts = np.bincount(both.ravel(), minlength=FT_IN)
    usedrows = np.flatnonzero(counts)
    u = usedrows.size
    e = max(0, u - NA_CAP)
    if e:
        order = np.argsort(counts[usedrows], kind="stable")
        excess_rows = np.sort(usedrows[order[:e]])
        a_rows = np.sort(usedrows[order[e:]])
    else:
        excess_rows = np.empty(0, np.int64)
        a_rows = usedrows
    assert e <= BCAP, f"overflow region too small: {e} > {BCAP}"

    remap = np.zeros(FT_IN, np.int64)
    remap[a_rows] = np.arange(a_rows.size)
    remap[excess_rows] = BBASE + np.arange(e)
    dev = remap[both]                                   # [2, 1024, 32]
    dev.sort(axis=2)      # ascending: banded A ids first, overflow at tail

    w_dev = np.zeros((VDEV, F), dtype=F8_NP)
    w_dev[:a_rows.size] = (W_ft[a_rows] * TSCALE).astype(F8_NP)
    if e:
        w_dev[BBASE:BBASE + e] = (W_ft[excess_rows] * TSCALE).astype(F8_NP)

    spread1024 = (np.arange(2 * JH * P) * 7) % 64
    spread512 = (np.arange(JH * P) * 7) % 64
    avecs = []     # NPH*NGA vectors of 1024 ids
    bvecs = []     # NPH lists of per-slot 512-id vectors
    maxb = 0
    for ph in range(NPH):
        s, hf = divmod(ph, 2)
        seg = dev[s, hf * JH * P:(hf + 1) * JH * P, :]   # [512, 32]
        for g in range(NGA):
            vec = np.concatenate([seg[:, 2 * g], seg[:, 2 * g + 1]])
            vec = np.where(vec >= BBASE, AZ0 + spread1024, vec)
            avecs.append(vec)
        nB = (seg >= BBASE).sum(axis=1)
        maxb = max(maxb, int(nB.max()))
        cols = []
        for m in range(int(nB.max())):
            col = seg[np.arange(JH * P), K - 1 - m]
            cols.append(np.where(nB > m, col - BBASE, BZ0 + spread512))
        bvecs.append(cols)
    return w_dev, avecs, bvecs, maxb


def _kernel_np(stm_indices, nstm_indices, values, W_ft, b_ft, W_out, b_out):
    """Correct fallback for inputs the HW fast path doesn't cover."""
    stm_ft = np.einsum("bk,bkf->bf", values, W_ft[stm_indices]) + b_ft
    nstm_ft = np.einsum("bk,bkf->bf", values, W_ft[nstm_indices]) + b_ft
    hidden = np.clip(np.concatenate([stm_ft, nstm_ft], axis=1), 0.0, 1.0)
    return 1.0 / (1.0 + np.exp(-(hidden @ W_out + b_out)))


def kernel(stm_indices, nstm_indices, values, W_ft, b_ft, W_out, b_out,
           _trace=False):
    stm_indices = np.asarray(stm_indices)
    nstm_indices = np.asarray(nstm_indices)
    values = np.asarray(values, dtype=np.float32)
    W_ft = np.ascontiguousarray(np.asarray(W_ft, dtype=np.float32))
    b_ft = np.asarray(b_ft, dtype=np.float32)
    W_out = np.asarray(W_out, dtype=np.float32)
    b_out = np.asarray(b_out, dtype=np.float32)

    if not np.all(values == 1.0):
        r = _kernel_np(stm_indices, nstm_indices, values, W_ft, b_ft,
                       W_out, b_out)
        return (r, None) if _trace else r

    preps = []
    nbslots = 0
    for c in range(NCORES):
        sl = slice(c * BPC, (c + 1) * BPC)
        p = _prep_core(stm_indices[sl], nstm_indices[sl], W_ft)
        nbslots = max(nbslots, p[3])
        preps.append(p)

    nc = _get(nbslots)

    bias_rep = np.ascontiguousarray(
        np.broadcast_to(b_ft, (P, JH, F)).astype(BF16_NP))
    wout_rep = np.ascontiguousarray(
        np.broadcast_to(W_out[:, 0].reshape(2, F)[None, :, :],
                        (P, 2, F)).astype(BF16_NP))
    bout_rep = np.full((P, 1), b_out[0], dtype=np.float32)

    in_maps = []
    for c in range(NCORES):
        w_dev, avecs, bvecs, _ = preps[c]
        idxa = np.stack([_wrap16(v) for v in avecs], axis=1)  # [128, 64, 64]
        m = {
            "w_ft": w_dev,
            "idxa": np.ascontiguousarray(idxa),
            "bias": bias_rep,
            "wout": wout_rep,
            "bout": bout_rep,
        }
        if nbslots:
            idxb = np.zeros((P, NPH, nbslots * 32), np.int16)
            for ph in range(NPH):
                for mi in range(nbslots):
                    if mi < len(bvecs[ph]):
                        vec = bvecs[ph][mi]
                    else:
                        vec = BZ0 + (np.arange(JH * P) * 7) % 64
                    idxb[:, ph, mi * 32:(mi + 1) * 32] = _wrap16(vec)
            m["idxb"] = np.ascontiguousarray(idxb)
        in_maps.append(m)

    res = run_bass_kernel_spmd(
        nc, in_maps, core_ids=list(range(NCORES)), trace=_trace
    )
    # out[p, j] holds batch row j*128 + p
    out = np.concatenate(
        [res.results[c]["out"].T.reshape(BPC) for c in range(NCORES)]
    ).reshape(8192, 1)
    if _trace:
        return out, res
    return out


# revision 25
# speedup vs baseline: 1.8549x; 1.0167x over previous
"""Trainium2 Bass kernel for nn_NnBoard768 (sparse embedding-lookup NNUE head).

Strategy (data-parallel over batch, 8 cores, input-specialized compile):
  - Each core handles 1024 of the 8192 batch rows; row b sits at SBUF
    partition b%128, free-slot b//128.
  - The feature table is REMAPPED per core: a core references only ~32.7k
    unique rows, so its table copy holds exactly those rows (fp8 e4m3,
    premultiplied by TSCALE) with device ids 0..nA-1 < 32704 — directly
    addressable by the int16 indices of the TIE `dma_gather` instruction.
    The (rare, data-dependent) overflow rows live in a small "B" region
    addressed by a second gather view.  This removes the two-pass
    zero-row-junk scheme entirely: every gathered descriptor is a needed
    row, halving HBM gather traffic vs. the two-pass kernel.
  - Per-position indices are sorted ascending, so gather k covers a narrow
    band of table rows (HBM locality) and overflow ids cluster at the tail.
  - Accumulation over the 32 active features runs on the tensor engine with
    fp8 DoubleRow matmuls (2 gathered tiles per instruction, 0.5 cyc/row)
    against a stacked scaled identity; b_ft is added by one bf16 matmul.
  - Work is split into 4 PSUM phases (side x batch-half, 4 banks each,
    double-buffered) so each phase's DVE epilogue (clip to [0,1], fused
    multiply+reduce against W_out) overlaps the next phase's gathers.
"""

import sys

sys.path.insert(0, "/opt/trn_rl_repo")

import numpy as np
import ml_dtypes

from concourse import bacc, bass, mybir
import concourse.tile as tile
from concourse.bass_utils import run_bass_kernel_spmd

P = 128          # SBUF partitions
K = 32           # nnz (active features per position)
J = 8            # batch slots per partition per core
JH = 4           # j-blocks per PSUM phase
F = 512          # feature-table output width
NCORES = 8
BPC = P * J      # batch rows per core (1024)
FT_IN = 40960

BBASE = 32768            # overflow ("B") region base device row
BCAP = 576               # max overflow rows
BZ0 = 576                # B-junk ids 576..639 (zero rows)
VDEV_B = BBASE + BCAP + 64  # device rows when an overflow region is needed

f32 = mybir.dt.float32
bf16 = mybir.dt.bfloat16
i16 = mybir.dt.int16
f8 = mybir.dt.float8e4
F8_NP = ml_dtypes.float8_e4m3
BF16_NP = ml_dtypes.bfloat16
Alu = mybir.AluOpType
DR = mybir.MatmulPerfMode.DoubleRow

TSCALE = 64.0    # host premultiplier; PE identity = 1/TSCALE
NQ = 4           # SWDGE descriptor-generation queues
GBUFS = 8
NPH = 4          # phases: (side, half)
NKG = 2          # k-slots per A-gather (1024-descriptor HW cap)
NGA = K // NKG   # A-gathers per phase
NIDX = NKG * JH * P         # idxs per A-gather (1024)
SA16 = NIDX // 16           # 64 idx cols per A-gather

GBUFS_A = 12     # ring depth for the 4KB/partition A-gather tiles


def _build(nbslots: int):
    nc = bacc.Bacc("TRN2", target_bir_lowering=False, debug=False,
                   num_devices=NCORES, num_swdge_queues=NQ)

    vdev = VDEV_B if nbslots else BBASE
    wft = nc.dram_tensor("w_ft", [vdev, F], f8, kind="ExternalInput")
    idxa_in = nc.dram_tensor("idxa", [P, NPH * NGA, SA16], i16,
                             kind="ExternalInput")
    if nbslots:
        idxb_in = nc.dram_tensor("idxb", [P, NPH, nbslots * 32], i16,
                                 kind="ExternalInput")
    bias_in = nc.dram_tensor("bias", [P, JH, F], bf16, kind="ExternalInput")
    wout_in = nc.dram_tensor("wout", [P, 2, F], bf16, kind="ExternalInput")
    bout_in = nc.dram_tensor("bout", [P, 1], f32, kind="ExternalInput")
    idw_in = nc.dram_tensor("idw", [P, 2, P], f8, kind="ExternalInput")
    idb_in = nc.dram_tensor("idb", [P, P], bf16, kind="ExternalInput")
    out = nc.dram_tensor("out", [P, J], f32, kind="ExternalOutput")

    qn = 0
    with tile.TileContext(nc) as tc:
        with tc.tile_pool(name="sbuf", bufs=1) as pool, \
             tc.tile_pool(name="gather", bufs=GBUFS) as gpool, \
             tc.tile_pool(name="psum", bufs=2, space="PSUM") as ppool:
            idxa = pool.tile([P, NPH * NGA, SA16], i16, tag="idxa")
            nc.sync.dma_start(out=idxa[:], in_=idxa_in[:])
            if nbslots:
                idxb = pool.tile([P, NPH, nbslots * 32], i16, tag="idxb")
                nc.sync.dma_start(out=idxb[:], in_=idxb_in[:])
            bias_sb = pool.tile([P, JH, F], bf16, tag="bias")
            nc.sync.dma_start(out=bias_sb[:], in_=bias_in[:])
            wout_sb = pool.tile([P, 2, F], bf16, tag="wout")
            nc.sync.dma_start(out=wout_sb[:], in_=wout_in[:])
            bout_sb = pool.tile([P, 1], f32, tag="bout")
            nc.sync.dma_start(out=bout_sb[:], in_=bout_in[:])

            identW = pool.tile([P, 2, P], f8, tag="identW")
            nc.sync.dma_start(out=identW[:], in_=idw_in[:])
            identB = pool.tile([P, P], bf16, tag="identB")
            nc.sync.dma_start(out=identB[:], in_=idb_in[:])

            z = [pool.tile([P, J], f32, tag=f"z{s}", name=f"z{s}")
                 for s in range(2)]
            prod = pool.tile([P, JH, F], bf16, tag="prod")

            for ph in range(NPH):
                s, hf = divmod(ph, 2)
                acc = ppool.tile([P, JH, F], f32, tag="acc", name=f"acc{ph}")
                for g in range(NGA):
                    ga = gpool.tile([P, NKG, JH, F], f8, tag="ga",
                                    name=f"ga{ph}_{g}", bufs=GBUFS_A)
                    nc.gpsimd.dma_gather(
                        ga[:].rearrange("p i j f -> p (i j) f"),
                        wft[:, :], idxa[:, ph * NGA + g, :],
                        num_idxs=NIDX, num_idxs_reg=NIDX,
                        elem_size=F, queue_num=qn % NQ)
                    qn += 1
                    for kk in range(NKG // 2):
                        for jh in range(JH):
                            nc.tensor.matmul(
                                acc[:, jh, :], identW[:],
                                ga[:, 2 * kk:2 * kk + 2, jh, :],
                                start=(g == 0 and kk == 0), stop=False,
                                perf_mode=DR)
                if nbslots:
                    gb = gpool.tile([P, nbslots, JH, F], f8, tag="gb",
                                    name=f"gb{ph}")
                    nc.gpsimd.dma_gather(
                        gb[:].rearrange("p m j f -> p (m j) f"),
                        wft[BBASE:, :], idxb[:, ph, :],
                        num_idxs=nbslots * JH * P,
                        num_idxs_reg=nbslots * JH * P,
                        elem_size=F, queue_num=qn % NQ)
                    qn += 1
                    for m in range(nbslots):
                        for jh in range(JH):
                            nc.tensor.matmul(
                                acc[:, jh, :], identW[:, 0, :],
                                gb[:, m, jh, :],
                                start=False, stop=False)
                for jh in range(JH):
                    nc.tensor.matmul(acc[:, jh, :], identB[:],
                                     bias_sb[:, jh, :],
                                     start=False, stop=True)

                h = pool.tile([P, JH, F], bf16, tag="h", name=f"h{ph}")
                nc.vector.tensor_scalar(
                    out=h[:], in0=acc[:], scalar1=0.0, scalar2=1.0,
                    op0=Alu.max, op1=Alu.min)
                wb = (wout_sb[:, s, :]
                      .rearrange("p (j f) -> p j f", j=1)
                      .broadcast_to([P, JH, F]))
                nc.vector.tensor_tensor(out=prod[:], in0=h[:], in1=wb,
                                        op=Alu.mult)
                nc.vector.tensor_reduce(
                    out=z[s][:, hf * JH:(hf + 1) * JH], in_=prod[:],
                    axis=mybir.AxisListType.X, op=Alu.add)

            nc.vector.tensor_tensor(out=z[0][:], in0=z[0][:], in1=z[1][:],
                                    op=Alu.add)
            out_sb = pool.tile([P, J], f32, tag="out")
            nc.scalar.activation(
                out=out_sb[:], in_=z[0][:],
                func=mybir.ActivationFunctionType.Sigmoid,
                bias=bout_sb[:, :1])
            nc.sync.dma_start(out=out.ap(), in_=out_sb[:])

    nc.compile()
    return nc


_cache = {}


def _get(nbslots: int):
    if nbslots not in _cache:
        _cache[nbslots] = _build(nbslots)
    return _cache[nbslots]


def _wrap16(vec: np.ndarray) -> np.ndarray:
    """int vector -> [128, len//16] int16 tile (idx i at partition i%16,
    col i//16, replicated across the 8 16-partition groups)."""
    arr = vec.reshape(-1, 16).T.astype(np.int16)
    return np.tile(arr, (8, 1))


def _rebalance(stm: np.ndarray, nstm: np.ndarray):
    """Assign batch rows to cores so every core references <= 32768 unique
    table rows (then int16 gather ids cover them directly, no overflow
    pass).  Greedy pair swaps between the worst and best cores."""
    draws = np.concatenate([stm, nstm], axis=1).astype(np.int64)  # [8192, 64]
    rows = np.arange(NCORES * BPC).reshape(NCORES, BPC)
    cnts = [np.bincount(draws[rows[c]].ravel(), minlength=FT_IN)
            for c in range(NCORES)]
    for _ in range(256):
        us = np.array([(c > 0).sum() for c in cnts])
        if us.max() <= BBASE:
            return rows, True
        cmax, cmin = int(us.argmax()), int(us.argmin())
        dmax = draws[rows[cmax]]
        excl = (cnts[cmax][dmax] == 1).sum(axis=1)
        ra = int(excl.argmax())
        dmin = draws[rows[cmin]]
        newu = (cnts[cmax][dmin] == 0).sum(axis=1)
        rb = int(newu.argmin())
        ga, gb = rows[cmax][ra], rows[cmin][rb]
        np.subtract.at(cnts[cmax], draws[ga], 1)
        np.add.at(cnts[cmax], draws[gb], 1)
        np.subtract.at(cnts[cmin], draws[gb], 1)
        np.add.at(cnts[cmin], draws[ga], 1)
        rows[cmax][ra], rows[cmin][rb] = gb, ga
    us = np.array([(c > 0).sum() for c in cnts])
    return rows, bool(us.max() <= BBASE)


def _prep_core(stm_c: np.ndarray, nstm_c: np.ndarray, W_ft: np.ndarray,
               cap: int):
    """Remap one core's referenced table rows into the device layout and
    build its gather index vectors.

    Returns (w_dev fp8 table, avecs (NPH*NGA vectors of NIDX ids),
    bvecs (per phase: per-overflow-slot 512-id vectors), maxb)."""
    both = np.stack([stm_c, nstm_c]).astype(np.int64)   # [2, 1024, 32]
    counts = np.bincount(both.ravel(), minlength=FT_IN)
    usedrows = np.flatnonzero(counts)
    u = usedrows.size
    e = max(0, u - cap)
    if e:
        order = np.argsort(counts[usedrows], kind="stable")
        excess_rows = np.sort(usedrows[order[:e]])
        a_rows = np.sort(usedrows[order[e:]])
    else:
        excess_rows = np.empty(0, np.int64)
        a_rows = usedrows
    assert e <= BCAP, f"overflow region too small: {e} > {BCAP}"

    remap = np.zeros(FT_IN, np.int64)
    remap[a_rows] = np.arange(a_rows.size)
    remap[excess_rows] = BBASE + np.arange(e)
    dev = remap[both]                                   # [2, 1024, 32]
    dev.sort(axis=2)      # ascending: banded A ids first, overflow at tail

    vdev = VDEV_B if e else BBASE
    w_dev = np.zeros((vdev, F), dtype=F8_NP)
    w_dev[:a_rows.size] = (W_ft[a_rows] * TSCALE).astype(F8_NP)
    if e:
        w_dev[BBASE:BBASE + e] = (W_ft[excess_rows] * TSCALE).astype(F8_NP)

    spreadA = cap + (np.arange(NIDX) * 7) % 64
    spread512 = BZ0 + (np.arange(JH * P) * 7) % 64
    avecs = []     # NPH*NGA vectors of NIDX ids
    bvecs = []     # NPH lists of per-overflow-slot 512-id vectors
    maxb = 0
    for ph in range(NPH):
        s, hf = divmod(ph, 2)
        seg = dev[s, hf * JH * P:(hf + 1) * JH * P, :]   # [512, 32]
        segr = seg.reshape(JH, P, K)     # [jh, p, k]
        for g in range(NGA):
            # descriptor b*128+p -> (k = NKG*g + b//JH, jh = b%JH, p)
            vec = np.transpose(
                segr[:, :, NKG * g:NKG * (g + 1)], (2, 0, 1)).ravel()
            vec = np.where(vec >= BBASE, spreadA, vec)
            avecs.append(vec)
        nB = (seg >= BBASE).sum(axis=1)
        maxb = max(maxb, int(nB.max()))
        cols = []
        for m in range(int(nB.max())):
            col = seg[np.arange(JH * P), K - 1 - m]
            cols.append(np.where(nB > m, col - BBASE, spread512))
        bvecs.append(cols)
    return w_dev, avecs, bvecs, maxb


def _kernel_np(stm_indices, nstm_indices, values, W_ft, b_ft, W_out, b_out):
    """Correct fallback for inputs the HW fast path doesn't cover."""
    stm_ft = np.einsum("bk,bkf->bf", values, W_ft[stm_indices]) + b_ft
    nstm_ft = np.einsum("bk,bkf->bf", values, W_ft[nstm_indices]) + b_ft
    hidden = np.clip(np.concatenate([stm_ft, nstm_ft], axis=1), 0.0, 1.0)
    return 1.0 / (1.0 + np.exp(-(hidden @ W_out + b_out)))


def kernel(stm_indices, nstm_indices, values, W_ft, b_ft, W_out, b_out,
           _trace=False):
    stm_indices = np.asarray(stm_indices)
    nstm_indices = np.asarray(nstm_indices)
    values = np.asarray(values, dtype=np.float32)
    W_ft = np.ascontiguousarray(np.asarray(W_ft, dtype=np.float32))
    b_ft = np.asarray(b_ft, dtype=np.float32)
    W_out = np.asarray(W_out, dtype=np.float32)
    b_out = np.asarray(b_out, dtype=np.float32)

    if not np.all(values == 1.0):
        r = _kernel_np(stm_indices, nstm_indices, values, W_ft, b_ft,
                       W_out, b_out)
        return (r, None) if _trace else r

    rows8, balanced = _rebalance(stm_indices, nstm_indices)
    cap = BBASE if balanced else BBASE - 64
    preps = []
    nbslots = 0
    for c in range(NCORES):
        rc = rows8[c]
        p = _prep_core(stm_indices[rc], nstm_indices[rc], W_ft, cap)
        nbslots = max(nbslots, p[3])
        preps.append(p)

    nc = _get(nbslots)
    vdev = VDEV_B if nbslots else BBASE

    bias_rep = np.ascontiguousarray(
        np.broadcast_to(b_ft, (P, JH, F)).astype(BF16_NP))
    wout_rep = np.ascontiguousarray(
        np.broadcast_to(W_out[:, 0].reshape(2, F)[None, :, :],
                        (P, 2, F)).astype(BF16_NP))
    bout_rep = np.full((P, 1), b_out[0], dtype=np.float32)
    idw = np.zeros((P, 2, P), dtype=F8_NP)
    idw[:, 0, :] = idw[:, 1, :] = (np.eye(P) / TSCALE).astype(F8_NP)
    idb = np.ascontiguousarray(np.eye(P).astype(BF16_NP))

    in_maps = []
    for c in range(NCORES):
        w_dev, avecs, bvecs, _ = preps[c]
        if w_dev.shape[0] != vdev:
            w_full = np.zeros((vdev, F), dtype=F8_NP)
            w_full[:w_dev.shape[0]] = w_dev
            w_dev = w_full
        idxa = np.stack([_wrap16(v) for v in avecs], axis=1)  # [128, 64, 64]
        m = {
            "w_ft": w_dev,
            "idxa": np.ascontiguousarray(idxa),
            "bias": bias_rep,
            "wout": wout_rep,
            "bout": bout_rep,
            "idw": idw,
            "idb": idb,
        }
        if nbslots:
            idxb = np.zeros((P, NPH, nbslots * 32), np.int16)
            for ph in range(NPH):
                for mi in range(nbslots):
                    if mi < len(bvecs[ph]):
                        vec = bvecs[ph][mi]
                    else:
                        vec = BZ0 + (np.arange(JH * P) * 7) % 64
                    idxb[:, ph, mi * 32:(mi + 1) * 32] = _wrap16(vec)
            m["idxb"] = np.ascontiguousarray(idxb)
        in_maps.append(m)

    res = run_bass_kernel_spmd(
        nc, in_maps, core_ids=list(range(NCORES)), trace=_trace
    )
    # core c's position j*128 + p holds batch row rows8[c][j*128 + p]
    out = np.zeros((NCORES * BPC, 1), dtype=np.float32)
    for c in range(NCORES):
        out[rows8[c], 0] = res.results[c]["out"].T.reshape(BPC)
    if _trace:
        return out, res
    return out


# revision 29
# speedup vs baseline: 1.8777x; 1.0123x over previous
"""Trainium2 Bass kernel for nn_NnBoard768 (sparse embedding-lookup NNUE head).

Strategy (data-parallel over batch, 8 cores, input-specialized compile):
  - Each core handles 1024 of the 8192 batch rows; row b sits at SBUF
    partition b%128, free-slot b//128.
  - The feature table is REMAPPED per core: a core references only ~32.7k
    unique rows, so its table copy holds exactly those rows (fp8 e4m3,
    premultiplied by TSCALE) with device ids 0..nA-1 < 32704 — directly
    addressable by the int16 indices of the TIE `dma_gather` instruction.
    The (rare, data-dependent) overflow rows live in a small "B" region
    addressed by a second gather view.  This removes the two-pass
    zero-row-junk scheme entirely: every gathered descriptor is a needed
    row, halving HBM gather traffic vs. the two-pass kernel.
  - Per-position indices are sorted ascending, so gather k covers a narrow
    band of table rows (HBM locality) and overflow ids cluster at the tail.
  - Accumulation over the 32 active features runs on the tensor engine with
    fp8 DoubleRow matmuls (2 gathered tiles per instruction, 0.5 cyc/row)
    against a stacked scaled identity; b_ft is added by one bf16 matmul.
  - Work is split into 4 PSUM phases (side x batch-half, 4 banks each,
    double-buffered) so each phase's DVE epilogue (clip to [0,1], fused
    multiply+reduce against W_out) overlaps the next phase's gathers.
"""

import sys

sys.path.insert(0, "/opt/trn_rl_repo")

import numpy as np
import ml_dtypes

from concourse import bacc, bass, mybir
import concourse.tile as tile
from concourse.bass_utils import run_bass_kernel_spmd

P = 128          # SBUF partitions
K = 32           # nnz (active features per position)
J = 8            # batch slots per partition per core
JH = 4           # j-blocks per PSUM phase
F = 512          # feature-table output width
NCORES = 8
BPC = P * J      # batch rows per core (1024)
FT_IN = 40960

BBASE = 32768            # overflow ("B") region base device row
BCAP = 576               # max overflow rows
BZ0 = 576                # B-junk ids 576..639 (zero rows)
VDEV_B = BBASE + BCAP + 64  # device rows when an overflow region is needed

f32 = mybir.dt.float32
bf16 = mybir.dt.bfloat16
i16 = mybir.dt.int16
f8 = mybir.dt.float8e4
F8_NP = ml_dtypes.float8_e4m3
BF16_NP = ml_dtypes.bfloat16
Alu = mybir.AluOpType
DR = mybir.MatmulPerfMode.DoubleRow

TSCALE = 64.0    # host premultiplier; PE identity = 1/TSCALE
NQ = 4           # SWDGE descriptor-generation queues
GBUFS = 8
NPH = 4          # phases: (side, half)
NKG = 2          # k-slots per A-gather (1024-descriptor HW cap)
NGA = K // NKG   # A-gathers per phase
NIDX = NKG * JH * P         # idxs per A-gather (1024)
SA16 = NIDX // 16           # 64 idx cols per A-gather

GBUFS_A = 12     # ring depth for the 4KB/partition A-gather tiles


def _build(nbslots: int):
    nc = bacc.Bacc("TRN2", target_bir_lowering=False, debug=False,
                   num_devices=NCORES, num_swdge_queues=NQ)

    vdev = VDEV_B if nbslots else BBASE
    wft = nc.dram_tensor("w_ft", [vdev, F], f8, kind="ExternalInput")
    idxa_in = [
        nc.dram_tensor(f"idxa{ph}", [P, NGA, SA16], i16, kind="ExternalInput")
        for ph in range(NPH)
    ]
    if nbslots:
        idxb_in = nc.dram_tensor("idxb", [P, NPH, nbslots * 32], i16,
                                 kind="ExternalInput")
    bias_in = nc.dram_tensor("bias", [P, JH, F], bf16, kind="ExternalInput")
    wout_in = nc.dram_tensor("wout", [P, 2, F], bf16, kind="ExternalInput")
    bout_in = nc.dram_tensor("bout", [P, 1], f32, kind="ExternalInput")
    idw_in = nc.dram_tensor("idw", [P, 2, P], f8, kind="ExternalInput")
    idb_in = nc.dram_tensor("idb", [P, P], bf16, kind="ExternalInput")
    out = nc.dram_tensor("out", [P, J], f32, kind="ExternalOutput")

    qn = 0
    with tile.TileContext(nc) as tc:
        with tc.tile_pool(name="sbuf", bufs=1) as pool, \
             tc.tile_pool(name="gather", bufs=GBUFS) as gpool, \
             tc.tile_pool(name="psum", bufs=2, space="PSUM") as ppool:
            idxa = []
            for ph in range(NPH):
                t = pool.tile([P, NGA, SA16], i16, tag=f"idxa{ph}",
                              name=f"idxa{ph}")
                idxa.append(t)
            # phase-0 indices first so gathers can start immediately
            nc.sync.dma_start(out=idxa[0][:], in_=idxa_in[0][:])
            identW = pool.tile([P, 2, P], f8, tag="identW")
            nc.sync.dma_start(out=identW[:], in_=idw_in[:])
            for ph in range(1, NPH):
                nc.sync.dma_start(out=idxa[ph][:], in_=idxa_in[ph][:])
            if nbslots:
                idxb = pool.tile([P, NPH, nbslots * 32], i16, tag="idxb")
                nc.sync.dma_start(out=idxb[:], in_=idxb_in[:])
            bias_sb = pool.tile([P, JH, F], bf16, tag="bias")
            nc.sync.dma_start(out=bias_sb[:], in_=bias_in[:])
            wout_sb = pool.tile([P, 2, F], bf16, tag="wout")
            nc.sync.dma_start(out=wout_sb[:], in_=wout_in[:])
            bout_sb = pool.tile([P, 1], f32, tag="bout")
            nc.sync.dma_start(out=bout_sb[:], in_=bout_in[:])
            identB = pool.tile([P, P], bf16, tag="identB")
            nc.sync.dma_start(out=identB[:], in_=idb_in[:])

            z = [pool.tile([P, J], f32, tag=f"z{s}", name=f"z{s}")
                 for s in range(2)]
            prod = pool.tile([P, JH, F], bf16, tag="prod")

            for ph in range(NPH):
                s, hf = divmod(ph, 2)
                acc = ppool.tile([P, JH, F], f32, tag="acc", name=f"acc{ph}")
                for g in range(NGA):
                    ga = gpool.tile([P, NKG, JH, F], f8, tag="ga",
                                    name=f"ga{ph}_{g}", bufs=GBUFS_A)
                    nc.gpsimd.dma_gather(
                        ga[:].rearrange("p i j f -> p (i j) f"),
                        wft[:, :], idxa[ph][:, g, :],
                        num_idxs=NIDX, num_idxs_reg=NIDX,
                        elem_size=F, queue_num=qn % NQ)
                    qn += 1
                    for kk in range(NKG // 2):
                        for jh in range(JH):
                            nc.tensor.matmul(
                                acc[:, jh, :], identW[:],
                                ga[:, 2 * kk:2 * kk + 2, jh, :],
                                start=(g == 0 and kk == 0), stop=False,
                                perf_mode=DR)
                if nbslots:
                    gb = gpool.tile([P, nbslots, JH, F], f8, tag="gb",
                                    name=f"gb{ph}")
                    nc.gpsimd.dma_gather(
                        gb[:].rearrange("p m j f -> p (m j) f"),
                        wft[BBASE:, :], idxb[:, ph, :],
                        num_idxs=nbslots * JH * P,
                        num_idxs_reg=nbslots * JH * P,
                        elem_size=F, queue_num=qn % NQ)
                    qn += 1
                    for m in range(nbslots):
                        for jh in range(JH):
                            nc.tensor.matmul(
                                acc[:, jh, :], identW[:, 0, :],
                                gb[:, m, jh, :],
                                start=False, stop=False)
                for jh in range(JH):
                    nc.tensor.matmul(acc[:, jh, :], identB[:],
                                     bias_sb[:, jh, :],
                                     start=False, stop=True)

                h = pool.tile([P, JH, F], bf16, tag="h", name=f"h{ph}")
                nc.vector.tensor_scalar(
                    out=h[:], in0=acc[:], scalar1=0.0, scalar2=1.0,
                    op0=Alu.max, op1=Alu.min)
                wb = (wout_sb[:, s, :]
                      .rearrange("p (j f) -> p j f", j=1)
                      .broadcast_to([P, JH, F]))
                nc.vector.tensor_tensor(out=prod[:], in0=h[:], in1=wb,
                                        op=Alu.mult)
                nc.vector.tensor_reduce(
                    out=z[s][:, hf * JH:(hf + 1) * JH], in_=prod[:],
                    axis=mybir.AxisListType.X, op=Alu.add)

            nc.vector.tensor_tensor(out=z[0][:], in0=z[0][:], in1=z[1][:],
                                    op=Alu.add)
            out_sb = pool.tile([P, J], f32, tag="out")
            nc.scalar.activation(
                out=out_sb[:], in_=z[0][:],
                func=mybir.ActivationFunctionType.Sigmoid,
                bias=bout_sb[:, :1])
            nc.sync.dma_start(out=out.ap(), in_=out_sb[:])

    nc.compile()
    return nc


_cache = {}


def _get(nbslots: int):
    if nbslots not in _cache:
        _cache[nbslots] = _build(nbslots)
    return _cache[nbslots]


def _wrap16(vec: np.ndarray) -> np.ndarray:
    """int vector -> [128, len//16] int16 tile (idx i at partition i%16,
    col i//16, replicated across the 8 16-partition groups)."""
    arr = vec.reshape(-1, 16).T.astype(np.int16)
    return np.tile(arr, (8, 1))


def _rebalance(stm: np.ndarray, nstm: np.ndarray):
    """Assign batch rows to cores so every core references <= 32768 unique
    table rows (then int16 gather ids cover them directly, no overflow
    pass).  Greedy pair swaps between the worst and best cores."""
    draws = np.concatenate([stm, nstm], axis=1).astype(np.int64)  # [8192, 64]
    rows = np.arange(NCORES * BPC).reshape(NCORES, BPC)
    cnts = [np.bincount(draws[rows[c]].ravel(), minlength=FT_IN)
            for c in range(NCORES)]
    for _ in range(256):
        us = np.array([(c > 0).sum() for c in cnts])
        if us.max() <= BBASE:
            return rows, True
        cmax, cmin = int(us.argmax()), int(us.argmin())
        dmax = draws[rows[cmax]]
        excl = (cnts[cmax][dmax] == 1).sum(axis=1)
        ra = int(excl.argmax())
        dmin = draws[rows[cmin]]
        newu = (cnts[cmax][dmin] == 0).sum(axis=1)
        rb = int(newu.argmin())
        ga, gb = rows[cmax][ra], rows[cmin][rb]
        np.subtract.at(cnts[cmax], draws[ga], 1)
        np.add.at(cnts[cmax], draws[gb], 1)
        np.subtract.at(cnts[cmin], draws[gb], 1)
        np.add.at(cnts[cmin], draws[ga], 1)
        rows[cmax][ra], rows[cmin][rb] = gb, ga
    us = np.array([(c > 0).sum() for c in cnts])
    return rows, bool(us.max() <= BBASE)


def _prep_core(stm_c: np.ndarray, nstm_c: np.ndarray, W_ft: np.ndarray,
               cap: int):
    """Remap one core's referenced table rows into the device layout and
    build its gather index vectors.

    Returns (w_dev fp8 table, avecs (NPH*NGA vectors of NIDX ids),
    bvecs (per phase: per-overflow-slot 512-id vectors), maxb)."""
    both = np.stack([stm_c, nstm_c]).astype(np.int64)   # [2, 1024, 32]
    counts = np.bincount(both.ravel(), minlength=FT_IN)
    usedrows = np.flatnonzero(counts)
    u = usedrows.size
    e = max(0, u - cap)
    if e:
        order = np.argsort(counts[usedrows], kind="stable")
        excess_rows = np.sort(usedrows[order[:e]])
        a_rows = np.sort(usedrows[order[e:]])
    else:
        excess_rows = np.empty(0, np.int64)
        a_rows = usedrows
    assert e <= BCAP, f"overflow region too small: {e} > {BCAP}"

    remap = np.zeros(FT_IN, np.int64)
    remap[a_rows] = np.arange(a_rows.size)
    remap[excess_rows] = BBASE + np.arange(e)
    dev = remap[both]                                   # [2, 1024, 32]
    dev.sort(axis=2)      # ascending: banded A ids first, overflow at tail

    vdev = VDEV_B if e else BBASE
    w_dev = np.zeros((vdev, F), dtype=F8_NP)
    w_dev[:a_rows.size] = (W_ft[a_rows] * TSCALE).astype(F8_NP)
    if e:
        w_dev[BBASE:BBASE + e] = (W_ft[excess_rows] * TSCALE).astype(F8_NP)

    spreadA = cap + (np.arange(NIDX) * 7) % 64
    spread512 = BZ0 + (np.arange(JH * P) * 7) % 64
    avecs = []     # NPH*NGA vectors of NIDX ids
    bvecs = []     # NPH lists of per-overflow-slot 512-id vectors
    maxb = 0
    for ph in range(NPH):
        s, hf = divmod(ph, 2)
        seg = dev[s, hf * JH * P:(hf + 1) * JH * P, :]   # [512, 32]
        segr = seg.reshape(JH, P, K)     # [jh, p, k]
        for g in range(NGA):
            # descriptor b*128+p -> (k = NKG*g + b//JH, jh = b%JH, p)
            vec = np.transpose(
                segr[:, :, NKG * g:NKG * (g + 1)], (2, 0, 1)).ravel()
            vec = np.where(vec >= BBASE, spreadA, vec)
            avecs.append(vec)
        nB = (seg >= BBASE).sum(axis=1)
        maxb = max(maxb, int(nB.max()))
        cols = []
        for m in range(int(nB.max())):
            col = seg[np.arange(JH * P), K - 1 - m]
            cols.append(np.where(nB > m, col - BBASE, spread512))
        bvecs.append(cols)
    return w_dev, avecs, bvecs, maxb


def _kernel_np(stm_indices, nstm_indices, values, W_ft, b_ft, W_out, b_out):
    """Correct fallback for inputs the HW fast path doesn't cover."""
    stm_ft = np.einsum("bk,bkf->bf", values, W_ft[stm_indices]) + b_ft
    nstm_ft = np.einsum("bk,bkf->bf", values, W_ft[nstm_indices]) + b_ft
    hidden = np.clip(np.concatenate([stm_ft, nstm_ft], axis=1), 0.0, 1.0)
    return 1.0 / (1.0 + np.exp(-(hidden @ W_out + b_out)))


def kernel(stm_indices, nstm_indices, values, W_ft, b_ft, W_out, b_out,
           _trace=False):
    stm_indices = np.asarray(stm_indices)
    nstm_indices = np.asarray(nstm_indices)
    values = np.asarray(values, dtype=np.float32)
    W_ft = np.ascontiguousarray(np.asarray(W_ft, dtype=np.float32))
    b_ft = np.asarray(b_ft, dtype=np.float32)
    W_out = np.asarray(W_out, dtype=np.float32)
    b_out = np.asarray(b_out, dtype=np.float32)

    if not np.all(values == 1.0):
        r = _kernel_np(stm_indices, nstm_indices, values, W_ft, b_ft,
                       W_out, b_out)
        return (r, None) if _trace else r

    rows8, balanced = _rebalance(stm_indices, nstm_indices)
    cap = BBASE if balanced else BBASE - 64
    preps = []
    nbslots = 0
    for c in range(NCORES):
        rc = rows8[c]
        p = _prep_core(stm_indices[rc], nstm_indices[rc], W_ft, cap)
        nbslots = max(nbslots, p[3])
        preps.append(p)

    nc = _get(nbslots)
    vdev = VDEV_B if nbslots else BBASE

    bias_rep = np.ascontiguousarray(
        np.broadcast_to(b_ft, (P, JH, F)).astype(BF16_NP))
    wout_rep = np.ascontiguousarray(
        np.broadcast_to(W_out[:, 0].reshape(2, F)[None, :, :],
                        (P, 2, F)).astype(BF16_NP))
    bout_rep = np.full((P, 1), b_out[0], dtype=np.float32)
    idw = np.zeros((P, 2, P), dtype=F8_NP)
    idw[:, 0, :] = idw[:, 1, :] = (np.eye(P) / TSCALE).astype(F8_NP)
    idb = np.ascontiguousarray(np.eye(P).astype(BF16_NP))

    in_maps = []
    for c in range(NCORES):
        w_dev, avecs, bvecs, _ = preps[c]
        if w_dev.shape[0] != vdev:
            w_full = np.zeros((vdev, F), dtype=F8_NP)
            w_full[:w_dev.shape[0]] = w_dev
            w_dev = w_full
        m = {
            "w_ft": w_dev,
            "bias": bias_rep,
            "wout": wout_rep,
            "bout": bout_rep,
            "idw": idw,
            "idb": idb,
        }
        for ph in range(NPH):
            m[f"idxa{ph}"] = np.ascontiguousarray(np.stack(
                [_wrap16(v) for v in avecs[ph * NGA:(ph + 1) * NGA]], axis=1))
        if nbslots:
            idxb = np.zeros((P, NPH, nbslots * 32), np.int16)
            for ph in range(NPH):
                for mi in range(nbslots):
                    if mi < len(bvecs[ph]):
                        vec = bvecs[ph][mi]
                    else:
                        vec = BZ0 + (np.arange(JH * P) * 7) % 64
                    idxb[:, ph, mi * 32:(mi + 1) * 32] = _wrap16(vec)
            m["idxb"] = np.ascontiguousarray(idxb)
        in_maps.append(m)

    res = run_bass_kernel_spmd(
        nc, in_maps, core_ids=list(range(NCORES)), trace=_trace
    )
    # core c's position j*128 + p holds batch row rows8[c][j*128 + p]
    out = np.zeros((NCORES * BPC, 1), dtype=np.float32)
    for c in range(NCORES):
        out[rows8[c], 0] = res.results[c]["out"].T.reshape(BPC)
    if _trace:
        return out, res
    return out
